# revision 17
# baseline (speedup 1.0000x reference)
"""CapsuleLayer dynamic-routing kernel for 8 Trainium2 NeuronCores (v2, fp16).

Problem: x [64,2048,16], route_weights [32,2048,16,32] ->
  3-iteration routing -> out [32,64,1,1,32] (fp32).

Sharding: capsules (C=32) split 4-per-core across 8 cores; x replicated.
All 16-bit data is fp16 (bf16 fails the 2e-2 gate: routing logits are
precision-sensitive; fp16 sim rel-err ~8e-3).

Per-core structure (c = 4 local capsules, b = 64, rj = 32768):
  phase A : psA[(c,o),b] += w2[k].T @ xt2[k]        (PE, 256 chunks, fp16)
  V step  : psU[(h,b), n] = oT_c.T @ wt[(c,o), n]   (PE, K=32 row-packed)
            vs = f16(psU)  (ACT)  ;  U = vs * x2h   (DVE 2x)
            delta = cascade-add over j (DVE), logits lP[c] += delta
  softmax : per-half max/exp (ACT accum Z), cross-half combine via small
            PE transposes -> alpha0/alpha1/rz in [b,c] layout
  s step  : xe_c = xt2 * eT_c (DVE 2x); psS[(c,o),(h,b)] += w2[k,c].T @ xe_c
            (PE col-group packed); s = (a0*s0T + a1*s1T)*rz; squash.
"""
import os
import numpy as np

C, B, R, CIN, OUT = 32, 64, 2048, 16, 32
NCORES = 8
CLOC = C // NCORES          # 4 capsules per core
RJ = R * CIN                # 32768  (j,r) / (r,j) linear size
NK = RJ // 128              # 256 chunks of 128
RH = RJ // 2                # 16384 cols per half in x2h / wt-half

_CACHE = {}


def _build_program():
    from contextlib import ExitStack
    import concourse.bass as bass
    import concourse.bacc as bacc
    import concourse.tile as tile
    from concourse import mybir

    f32 = mybir.dt.float32
    f16 = mybir.dt.float16
    AL = mybir.AluOpType
    AF = mybir.ActivationFunctionType
    AX = mybir.AxisListType

    nc = bacc.Bacc(None, target_bir_lowering=False,
                   detect_race_conditions=not bool(int(os.environ.get("CAPS_NO_RACE", "0"))))
    n_loops = int(os.environ.get("CAPS_LOOPS", "1"))

    # ---- DRAM I/O ----
    w2 = nc.dram_tensor("w2", [128, NK * 128], f16, kind="ExternalInput")  # [p,(k,co)]
    xt2 = nc.dram_tensor("xt2", [128, NK * B], f16, kind="ExternalInput")  # [p,(k,b)]
    x2h = nc.dram_tensor("x2h", [128, RH], f16, kind="ExternalInput")    # [(h,b),(r,j)/2]
    wt = nc.dram_tensor("wt", [CLOC, OUT, RJ], f16, kind="ExternalInput")  # [c,o,(r,j)]
    ident = nc.dram_tensor("ident", [128, 128], f32, kind="ExternalInput")
    out3 = nc.dram_tensor("out3", [B, 128], f32, kind="ExternalOutput")  # [b,(c,o)]

    with tile.TileContext(nc) as tc, ExitStack() as ctx:
        const = ctx.enter_context(tc.tile_pool(name="const", bufs=1))
        small = ctx.enter_context(tc.tile_pool(name="small", bufs=2))
        outp = ctx.enter_context(tc.tile_pool(name="outp", bufs=2))
        wtp_p = ctx.enter_context(tc.tile_pool(name="wtp", bufs=3))
        vs_p = ctx.enter_context(tc.tile_pool(name="vsp", bufs=3))
        cas_p = ctx.enter_context(tc.tile_pool(name="cas", bufs=1))
        xe_p = ctx.enter_context(tc.tile_pool(name="xep", bufs=2))
        eP_p = ctx.enter_context(tc.tile_pool(name="ep", bufs=1))
        psU_p = ctx.enter_context(tc.tile_pool(name="psU", bufs=2, space="PSUM"))
        psS0_p = ctx.enter_context(tc.tile_pool(name="psS0", bufs=1, space="PSUM"))
        psS1_p = ctx.enter_context(tc.tile_pool(name="psS1", bufs=1, space="PSUM"))
        psT_p = ctx.enter_context(tc.tile_pool(name="psT", bufs=2, space="PSUM"))

        idn = const.tile([128, 128], f32, tag="ident", name="idn")
        nc.sync.dma_start(out=idn, in_=ident[:])
        zz16 = const.tile([128, 128], f16, tag="zz16", name="zz16")
        nc.vector.memset(zz16[:], 0.0)

        NG = 8
        KG = NK // NG
        w2g = []
        xt2g = []
        for g in range(NG):
            t_ = const.tile([128, KG, 128], f16, tag=f"w2sb{g}", name=f"w2_sb{g}")
            nc.sync.dma_start(
                out=t_, in_=w2[:, KG * 128 * g:KG * 128 * (g + 1)].rearrange(
                    "p (k co) -> p k co", k=KG))
            w2g.append(t_)
            t2 = const.tile([128, KG, B], f16, tag=f"xt2sb{g}", name=f"xt2_sb{g}")
            nc.sync.dma_start(
                out=t2, in_=xt2[:, KG * B * g:KG * B * (g + 1)].rearrange(
                    "p (k b) -> p k b", k=KG))
            xt2g.append(t2)
        x2h_sb = const.tile([128, RH], f16, tag="x2h", name="x2h_sb")
        nc.sync.dma_start(out=x2h_sb, in_=x2h[:])

        # logits per capsule [(h,b)=128, r-in-half=1024]
        lP = [const.tile([128, R // 2], f32, tag=f"lP{c}", name=f"lP{c}")
              for c in range(CLOC)]
        # transposed probs per capsule [r%128, t=16, b] (t: 0-7 h0 rb, 8-15 h1 rb)
        p2T = [const.tile([128, 16, B], f16, tag=f"p2T{c}", name=f"p2T{c}")
               for c in range(CLOC)]

        def squash(u_bT, scale_pow):
            """u_bT [64,(4c,32o)] f32: s = u*scale_pow; out = s*sqrt(n2)/(n2+1).
            Returns (o_i [64,128] f32, oT [128,64] f16)."""
            sq = small.tile([B, 128], f32, tag="sq", name="sq")
            nc.vector.scalar_tensor_tensor(
                out=sq, in0=u_bT, scalar=float(scale_pow * scale_pow),
                in1=u_bT, op0=AL.mult, op1=AL.mult)
            n2 = small.tile([B, CLOC], f32, tag="n2", name="n2")
            nc.vector.tensor_reduce(
                out=n2, in_=sq[:].rearrange("b (c o) -> b c o", c=CLOC),
                axis=AX.X, op=AL.add)
            rt = small.tile([B, CLOC], f32, tag="rt", name="rt")
            nc.scalar.activation(out=rt, in_=n2, func=AF.Sqrt)
            dn = small.tile([B, CLOC], f32, tag="dn", name="dn")
            nc.vector.tensor_scalar_add(out=dn, in0=n2, scalar1=1.0)
            rc = small.tile([B, CLOC], f32, tag="rc", name="rc")
            nc.vector.reciprocal(out=rc, in_=dn)
            f = small.tile([B, CLOC], f32, tag="f", name="f")
            nc.vector.tensor_mul(out=f, in0=rt, in1=rc)
            f2 = small.tile([B, CLOC], f32, tag="f2", name="f2")
            nc.vector.tensor_scalar_mul(out=f2, in0=f, scalar1=float(scale_pow))
            o_i = outp.tile([B, 128], f32, tag="oi", name="oi")
            f2b = bass.AP(tensor=f2[:].tensor, offset=f2[:].offset,
                          ap=[f2[:].ap[0], f2[:].ap[1], [0, OUT]])
            nc.vector.tensor_tensor(
                out=o_i[:].rearrange("b (c o) -> b c o", c=CLOC),
                in0=u_bT[:].rearrange("b (c o) -> b c o", c=CLOC),
                in1=f2b, op=AL.mult)
            psOT = psT_p.tile([128, 128], f32, tag="psT", name="psOT")
            nc.tensor.transpose(psOT[:, 0:B], o_i, idn[0:B, 0:B])
            oT = outp.tile([128, B], f16, tag="oT", name="oT")
            nc.scalar.copy(out=oT, in_=psOT[:, 0:B])
            return o_i, oT

        for _loop in range(n_loops):
            # ---------- Phase A: s1 = (1/R) sum_(j,r) x W ----------
            psA = psS0_p.tile([128, B], f32, tag="psS0", name="psA")
            for k in range(NK):
                nc.tensor.matmul(psA, w2g[k // KG][:, k % KG, :],
                                 xt2g[k // KG][:, k % KG, :],
                                 start=(k == 0), stop=(k == NK - 1))
            sA = small.tile([128, B], f32, tag="sA", name="sA")
            nc.scalar.copy(out=sA, in_=psA)
            psAT = psT_p.tile([128, 128], f32, tag="psT", name="psAT")
            nc.tensor.transpose(psAT[0:B, :], sA, idn)
            uT = small.tile([B, 128], f32, tag="uT", name="uT")
            nc.scalar.copy(out=uT, in_=psAT[0:B, :])
            out_i, oT = squash(uT, 1.0 / R)

            # ---------- Two routing boundaries ----------
            for it in (1, 2):
                # --- V + delta (per 1024-col psU unit, both halves stacked) ---
                for u in range(16):
                    wtp = wtp_p.tile([128, 2, 1024], f16, tag="wtp", name="wtp")
                    nc.sync.dma_start(
                        out=wtp,
                        in_=wt[:].rearrange("c o (h n) -> (c o) h n", h=2)[
                            :, :, 1024 * u:1024 * (u + 1)])
                    for c in range(CLOC):
                        psU = psU_p.tile([128, 1024], f32, tag="psU", name="psU")
                        for i in range(2):
                            sl = slice(512 * i, 512 * (i + 1))
                            nc.tensor.matmul(
                                psU[0:64, sl], oT[32 * c:32 * (c + 1), :],
                                wtp[32 * c:32 * (c + 1), 0, sl],
                                start=True, stop=True, tile_position=(32 * c, 0))
                            nc.tensor.matmul(
                                psU[64:128, sl], oT[32 * c:32 * (c + 1), :],
                                wtp[32 * c:32 * (c + 1), 1, sl],
                                start=True, stop=True, tile_position=(32 * c, 64))
                        vs = vs_p.tile([128, 64, CIN], f16, tag="vs", name="vs")
                        nc.scalar.copy(
                            out=vs,
                            in_=psU[:].rearrange("p (r j) -> p r j", j=CIN))
                        nc.vector.tensor_tensor(
                            out=vs, in0=vs,
                            in1=x2h_sb[:, 1024 * u:1024 * (u + 1)].rearrange(
                                "p (r j) -> p r j", j=CIN),
                            op=AL.mult)
                        ca = cas_p.tile([128, 64, 8], f16, tag="ca", name="ca")
                        nc.vector.tensor_tensor(out=ca, in0=vs[:, :, 0:8],
                                                in1=vs[:, :, 8:16], op=AL.add)
                        cb = cas_p.tile([128, 64, 4], f16, tag="cb", name="cb")
                        nc.vector.tensor_tensor(out=cb, in0=ca[:, :, 0:4],
                                                in1=ca[:, :, 4:8], op=AL.add)
                        cc = cas_p.tile([128, 64, 2], f16, tag="cc", name="cc")
                        nc.vector.tensor_tensor(out=cc, in0=cb[:, :, 0:2],
                                                in1=cb[:, :, 2:4], op=AL.add)
                        if it == 1:
                            nc.vector.tensor_tensor(
                                out=lP[c][:, 64 * u:64 * (u + 1)],
                                in0=cc[:, :, 0], in1=cc[:, :, 1], op=AL.add)
                        else:
                            dt = small.tile([128, 64], f32, tag="dt", name="dt")
                            nc.vector.tensor_tensor(out=dt, in0=cc[:, :, 0],
                                                    in1=cc[:, :, 1], op=AL.add)
                            nc.vector.tensor_add(
                                out=lP[c][:, 64 * u:64 * (u + 1)],
                                in0=lP[c][:, 64 * u:64 * (u + 1)], in1=dt)

                # --- softmax pieces: per-half e, Z; cross-half alpha/Z combine ---
                mq = small.tile([128, 8], f32, tag="mq", name="mq")
                mn = small.tile([128, CLOC], f32, tag="mn", name="mn")
                for c in range(CLOC):
                    nc.vector.tensor_reduce(out=mq[:, c:c + 1], in_=lP[c],
                                            axis=AX.X, op=AL.max)
                    nc.vector.tensor_scalar_mul(out=mn[:, c:c + 1],
                                                in0=mq[:, c:c + 1], scalar1=-1.0)
                    eP = eP_p.tile([128, R // 2], f32, tag="eP", name="eP")
                    nc.scalar.activation(out=eP, in_=lP[c], func=AF.Exp,
                                         bias=mn[:, c:c + 1], scale=1.0,
                                         accum_out=mq[:, 4 + c:5 + c])
                    for rb in range(8):
                        psT2 = psT_p.tile([128, 128], f32, tag="psT", name="psT2")
                        nc.tensor.transpose(
                            psT2, eP[:, 128 * rb:128 * (rb + 1)], idn)
                        nc.scalar.copy(out=p2T[c][:, rb, :], in_=psT2[:, 0:64])
                        nc.scalar.copy(out=p2T[c][:, 8 + rb, :], in_=psT2[:, 64:128])
                # cross-half combine of m and Z via transposes to [4,128]
                psM = psT_p.tile([128, 128], f32, tag="psT", name="psM")
                nc.tensor.transpose(psM[0:4, :], mq[:, 0:4], idn)
                mT = small.tile([CLOC, 128], f32, tag="mT", name="mT")
                nc.scalar.copy(out=mT, in_=psM[0:4, :])
                psZ = psT_p.tile([128, 128], f32, tag="psT", name="psZ")
                nc.tensor.transpose(psZ[0:4, :], mq[:, 4:8], idn)
                zT = small.tile([CLOC, 128], f32, tag="zT", name="zT")
                nc.scalar.copy(out=zT, in_=psZ[0:4, :])
                mc = small.tile([CLOC, 64], f32, tag="mc", name="mc")
                nc.vector.tensor_tensor(out=mc, in0=mT[:, 0:64],
                                        in1=mT[:, 64:128], op=AL.max)
                aa = small.tile([CLOC, 3, 64], f32, tag="aa", name="aa")
                for h in range(2):
                    dm = small.tile([CLOC, 64], f32, tag="dm", name="dm")
                    nc.vector.tensor_sub(out=dm, in0=mT[:, 64 * h:64 * (h + 1)],
                                         in1=mc)
                    nc.scalar.activation(out=aa[:, h, :], in_=dm, func=AF.Exp)
                z0 = small.tile([CLOC, 64], f32, tag="z0", name="z0")
                nc.vector.tensor_mul(out=z0, in0=zT[:, 0:64], in1=aa[:, 0, :])
                z1 = small.tile([CLOC, 64], f32, tag="z1", name="z1")
                nc.vector.tensor_mul(out=z1, in0=zT[:, 64:128], in1=aa[:, 1, :])
                zc = small.tile([CLOC, 64], f32, tag="zc", name="zc")
                nc.vector.tensor_add(out=zc, in0=z0, in1=z1)
                nc.vector.reciprocal(out=aa[:, 2, :], in_=zc)
                # transpose [4,3*64] -> per-b [64, (3,4)] in one shot:
                # aa rows=c(4), cols=(h/rz 3, b 64): transpose -> [(3,64)?? no:
                # transpose each [4,64] slice separately into ab [64, 3, 4]
                ab = small.tile([64, 3, CLOC], f32, tag="ab", name="ab")
                for s3 in range(3):
                    psa = psT_p.tile([128, 128], f32, tag="psT", name="psa")
                    nc.tensor.transpose(psa[0:64, 0:4], aa[:, s3, :], idn[0:4, 0:4])
                    nc.scalar.copy(out=ab[:, s3, :], in_=psa[0:64, 0:4])

                # --- xe + s matmuls (psS cols: [0:64] half0, [64:128] half1) ---
                psS = [psS0_p.tile([128, B], f32, tag="psS0", name="psS0i"),
                       psS1_p.tile([128, B], f32, tag="psS1", name="psS1i")]
                for h in range(2):
                    nc.tensor.matmul(psS[h], zz16, xt2g[0][:, 0, :],
                                     start=True, stop=False,
                                     skip_group_check=True)
                for j in range(CIN):
                    xes = []
                    for c in range(CLOC):
                        xe = xe_p.tile([128, 16, B], f16, tag=f"xe{c}", name=f"xe{c}")
                        nc.vector.tensor_tensor(
                            out=xe, in0=xt2g[j // 2][:, 16 * (j % 2):16 * (j % 2 + 1), :],
                            in1=p2T[c][:], op=AL.mult)
                        xes.append(xe)
                    for t in range(16):
                        k = 16 * j + t
                        h = t // 8
                        for c in range(CLOC):
                            nc.tensor.matmul(
                                psS[h][32 * c:32 * (c + 1), :],
                                w2g[k // KG][:, k % KG, 32 * c:32 * (c + 1)],
                                xes[c][:, t, :],
                                start=False, stop=(k == 240 + 8 * h + 7),
                                tile_position=(0, 32 * c), skip_group_check=True)
                # s = (a0*s0T + a1*s1T) * rz ; squash
                sc = small.tile([128, 2, B], f32, tag="sc", name="sc")
                nc.scalar.copy(out=sc[:, 0, :], in_=psS[0])
                nc.scalar.copy(out=sc[:, 1, :], in_=psS[1])
                psH = psT_p.tile([128, 128], f32, tag="psT", name="psH")
                nc.tensor.transpose(psH[0:64, :], sc[:, 0, :], idn)
                s0T = small.tile([64, 128], f32, tag="s0T", name="s0T")
                nc.scalar.copy(out=s0T, in_=psH[0:64, :])
                psH2 = psT_p.tile([128, 128], f32, tag="psT", name="psH2")
                nc.tensor.transpose(psH2[0:64, :], sc[:, 1, :], idn)
                s1T = small.tile([64, 128], f32, tag="s1T", name="s1T")
                nc.scalar.copy(out=s1T, in_=psH2[0:64, :])

                def bcast(col):
                    a = ab[:, col, :]
                    return bass.AP(tensor=a.tensor, offset=a.offset,
                                   ap=[a.ap[0], a.ap[1], [0, OUT]])
                nc.vector.tensor_tensor(
                    out=s0T[:].rearrange("b (c o) -> b c o", c=CLOC),
                    in0=s0T[:].rearrange("b (c o) -> b c o", c=CLOC),
                    in1=bcast(0), op=AL.mult)
                nc.vector.tensor_tensor(
                    out=s1T[:].rearrange("b (c o) -> b c o", c=CLOC),
                    in0=s1T[:].rearrange("b (c o) -> b c o", c=CLOC),
                    in1=bcast(1), op=AL.mult)
                nc.vector.tensor_add(out=s0T, in0=s0T, in1=s1T)
                nc.vector.tensor_tensor(
                    out=s0T[:].rearrange("b (c o) -> b c o", c=CLOC),
                    in0=s0T[:].rearrange("b (c o) -> b c o", c=CLOC),
                    in1=bcast(2), op=AL.mult)
                out_i, oT = squash(s0T, 1.0)

            nc.sync.dma_start(out=out3[:], in_=out_i)

    nc.finalize()
    return nc


def _get_program():
    if "nc" not in _CACHE:
        _CACHE["nc"] = _build_program()
    return _CACHE["nc"]


def make_in_maps(x, route_weights):
    x = np.ascontiguousarray(x, dtype=np.float32)
    W = np.ascontiguousarray(route_weights, dtype=np.float32)
    xt2 = np.ascontiguousarray(
        x.transpose(2, 1, 0).reshape(NK, 128, B).transpose(1, 0, 2)
        .reshape(128, NK * B)).astype(np.float16)                 # [p,(k,b)]
    xnat = x.reshape(B, RJ)                                       # [b,(r,j)]
    x2h = np.ascontiguousarray(
        np.concatenate([xnat[:, :RH], xnat[:, RH:]], axis=0)).astype(np.float16)
    ident = np.eye(128, dtype=np.float32)
    in_maps = []
    for core in range(NCORES):
        wc = W[CLOC * core:CLOC * (core + 1)]                     # [4,R,J,O]
        m = {"w2": np.ascontiguousarray(
                wc.transpose(2, 1, 0, 3).reshape(NK, 128, CLOC * OUT)
                .transpose(1, 0, 2).reshape(128, NK * 128)).astype(np.float16),
             "wt": np.ascontiguousarray(
                wc.transpose(0, 3, 1, 2).reshape(CLOC, OUT, RJ)).astype(np.float16),
             "xt2": xt2, "x2h": x2h, "ident": ident}
        in_maps.append(m)
    return in_maps


def kernel(x, route_weights):
    from concourse.bass_utils import run_bass_kernel_spmd

    in_maps = make_in_maps(x, route_weights)
    nc = _get_program()
    kw = {}
    if os.environ.get("CAPS_TRACE_DIR"):
        kw["tmpdir"] = os.environ["CAPS_TRACE_DIR"]
    res = run_bass_kernel_spmd(nc, in_maps, core_ids=list(range(NCORES)), **kw)
    if os.environ.get("CAPS_RESULT_STASH"):
        _CACHE["last_result"] = res

    out = np.empty((C, B, 1, 1, OUT), dtype=np.float32)
    for core in range(NCORES):
        o = res.results[core]["out3"].reshape(B, CLOC, OUT).transpose(1, 0, 2)
        out[CLOC * core:CLOC * (core + 1), :, 0, 0, :] = o
    return out
